# revision 1
# baseline (speedup 1.0000x reference)
"""Trainium2 Bass kernel for the dendritic-branch spiking FNN (DH_SFNN).

Model (per reference):
  branch_in = x @ W_in.T + b_in                  # (B,T,H*BR)
  per t:  i_d = beta*i_d + (1-beta)*branch_in_t  # beta = sigmoid(tau_n), (H,BR)
          v   = alpha*v + (1-alpha)*i_d.sum(br)  # alpha = sigmoid(tau_m), (H,)
          spike = (v >= 1); v -= spike; counts += spike
  out = counts @ W_out.T + b_out                 # (B,D_OUT)

Strategy: data-parallel over batch across 8 cores (32 rows each). On each
core: fp32 GEMM on PE into a padded-(br,j,h) layout; the per-branch IIR
filter runs as ONE fused tensor_tensor_scan per (m-tile, chunk) covering all
local batches (a zero in the d0 multiplier column at each batch's t=0 resets
the recurrence; cross-chunk carry is injected into u's first column); the
nonlinear spike/reset loop runs per timestep with 3 (uniform alpha) or 4
scalar_tensor_tensor/tensor_tensor ops on a negated state; spike counts are
read out with a small PE matmul against W_out.
"""

import sys

if "/opt/trn_rl_repo" not in sys.path:
    sys.path.insert(0, "/opt/trn_rl_repo")

from contextlib import ExitStack

import numpy as np

import concourse.bass as bass
import concourse.mybir as mybir
import concourse.tile as tile
from concourse import bacc

B, T, D_IN, H, BR, D_OUT = 256, 500, 700, 200, 2, 35
NCORES = 8
BL = B // NCORES          # local batch = 32
NK = 6                    # k-tiles; D_IN padded 700 -> 768 so every tile is 128
DP = NK * 128             # padded contraction dim (768)
M = 4                     # m-tiles, m=(br,j): o'' = m*128 + p, h = (m%2)*128+p
OP = M * 128              # padded output rows (512)
NJ = 2                    # h groups (j=0: h<128, j=1: h 128..199)


def _f32(a):
    return np.ascontiguousarray(a, dtype=np.float32)


def _build(T_, C_, alpha_uniform_val=None, reps=1, gemm="f16x3",
           phase="full"):
    """Build the single-core Bass program. alpha_uniform_val: python float if
    alpha is constant across neurons (enables fused v-update), else None.
    reps>1 wraps the whole computation in an on-device loop (benchmarking).
    gemm: "f32" (true fp32, 4-pass matmuls) or "f16x3" (fp16 split high/low,
    3 one-pass matmuls -- measured max err 2.3e-6 vs 8.7e-7 for fp32)."""
    NCH = T_ // C_
    BG = 4                 # batches per matmul n-group
    NG = BL // BG          # 8 n-groups
    NN = BG * C_           # matmul free dim (<=512 for fp32)
    assert NN <= 512 and T_ % C_ == 0
    fp32 = mybir.dt.float32
    AF = mybir.ActivationFunctionType
    AL = mybir.AluOpType

    nc = bacc.Bacc("TRN2", target_bir_lowering=False, debug=False,
                   num_devices=NCORES)

    fp16 = mybir.dt.float16
    xdt = fp32 if gemm == "f32" else fp16
    nxs = 1 if gemm == "f32" else 2         # x/w operand copies (hi, lo)
    xt_d = nc.dram_tensor("xt", [nxs, T_ // C_, BL // 4, 128, NK * 4 * C_],
                          xdt, kind="ExternalInput")
    wt_d = nc.dram_tensor("wt", [nxs, NK, 128, OP], xdt, kind="ExternalInput")
    sc2_d = nc.dram_tensor("sc2", [128, M], fp32, kind="ExternalInput")
    b2_d = nc.dram_tensor("b2", [128, M], fp32, kind="ExternalInput")
    bt_d = nc.dram_tensor("bt", [128, M], fp32, kind="ExternalInput")
    a1c_d = nc.dram_tensor("a1c", [128, NJ], fp32, kind="ExternalInput")
    atile_d = nc.dram_tensor("atile", [128, NJ * BL], fp32, kind="ExternalInput")
    d0_d = nc.dram_tensor("d0", [M, 128, BL * C_], fp32, kind="ExternalInput")
    woutT_d = nc.dram_tensor("woutT", [2 * 128, D_OUT], fp32, kind="ExternalInput")
    bout_d = nc.dram_tensor("bout", [D_OUT, 1], fp32, kind="ExternalInput")

    out_d = nc.dram_tensor("out", [D_OUT, BL], fp32, kind="ExternalOutput")
    counts_d = nc.dram_tensor("counts", [128, NJ * BL], fp32, kind="ExternalOutput")
    # tiny passthrough tensor so benchmark harnesses can chain executions
    tok_d = nc.dram_tensor("tok", [1, 16], fp32, kind="ExternalInput")
    tok_o = nc.dram_tensor("tok_out", [1, 16], fp32, kind="ExternalOutput")

    with tile.TileContext(nc) as tc, ExitStack() as ctx:
        const = ctx.enter_context(tc.tile_pool(name="const", bufs=1))
        st = ctx.enter_context(tc.tile_pool(name="state", bufs=1))
        bigu = ctx.enter_context(tc.tile_pool(name="bigu", bufs=1))
        wp = ctx.enter_context(tc.tile_pool(name="wph", bufs=1))
        xp = ctx.enter_context(tc.tile_pool(name="xin", bufs=2))
        d0p = ctx.enter_context(tc.tile_pool(name="d0p", bufs=2))
        ps = ctx.enter_context(tc.tile_pool(name="psum", bufs=4, space="PSUM"))
        pso = ctx.enter_context(tc.tile_pool(name="psout", bufs=1, space="PSUM"))
        scr = ctx.enter_context(tc.tile_pool(name="scr", bufs=2))

        w_sbs = []
        for s in range(nxs):
            w_sb_s = const.tile([128, NK * OP], xdt, tag=f"wsb{s}")
            nc.sync.dma_start(
                w_sb_s[:].rearrange("p (k o) -> p k o", k=NK),
                wt_d.ap()[s].rearrange("k p o -> p k o"))
            w_sbs.append(w_sb_s)
        sc2 = const.tile([128, M], fp32)
        nc.sync.dma_start(sc2[:], sc2_d.ap())
        b2 = const.tile([128, M], fp32)
        nc.sync.dma_start(b2[:], b2_d.ap())
        bt = const.tile([128, M], fp32)
        nc.sync.dma_start(bt[:], bt_d.ap())
        a1c = const.tile([128, NJ], fp32)
        nc.sync.dma_start(a1c[:], a1c_d.ap())
        atile = const.tile([128, NJ * BL], fp32)
        nc.sync.dma_start(atile[:], atile_d.ap())
        woutT_sb = const.tile([128, 2 * D_OUT], fp32)
        nc.sync.dma_start(woutT_sb[:, 0:D_OUT], woutT_d.ap()[0:128])
        nc.sync.dma_start(woutT_sb[:, D_OUT:2 * D_OUT], woutT_d.ap()[128:256])
        bout_sb = const.tile([D_OUT, 1], fp32)
        nc.sync.dma_start(bout_sb[:], bout_d.ap())

        def body_once():
            vst = st.tile([128, NJ * BL], fp32, tag="vst")  # negated potential
            counts = st.tile([128, NJ * BL], fp32, tag="cnt")
            carry = st.tile([128, M * BL], fp32, tag="carry")
            hist = st.tile([128, C_ * NJ * BL], fp32, tag="hist")
            nc.vector.memset(vst[:], 0.0)
            nc.vector.memset(counts[:], 0.0)
            u = bigu.tile([128, M * BL * C_], fp32, tag="u")
            wti = wp.tile([128, NJ * BL * C_], fp32, tag="wti")
            NF = NJ * BL

            for c in range(NCH):
                # -- GEMM: u[m-tile, b, t] = x @ W' (+bias, (1-beta)-scaled) --
                for g in range(NG):
                    x_sbs = []
                    for s in range(nxs):
                        x_sb_s = xp.tile([128, NK * NN], xdt, tag=f"xsb{s}")
                        nc.sync.dma_start(x_sb_s[:], xt_d.ap()[s, c, g])
                        x_sbs.append(x_sb_s)
                    # operand pairs: fp32 -> [(w,x)]; f16x3 -> hh, hl, lh
                    pairs = ([(0, 0)] if gemm == "f32"
                             else [(0, 0), (0, 1), (1, 0)])
                    for m in range(M):
                        pt = ps.tile([128, NN], fp32, tag="pt")
                        nmm = len(pairs) * NK
                        i = 0
                        for (ws, xs) in pairs:
                            for k in range(NK):
                                nc.tensor.matmul(
                                    pt[:],
                                    w_sbs[ws][:, k * OP + m * 128:
                                              k * OP + (m + 1) * 128],
                                    x_sbs[xs][:, k * NN:(k + 1) * NN],
                                    start=(i == 0), stop=(i == nmm - 1))
                                i += 1
                        nc.scalar.activation(
                            u[:, m * BL * C_ + g * NN: m * BL * C_ + (g + 1) * NN],
                            pt[:], AF.Identity,
                            bias=b2[:, m:m + 1], scale=sc2[:, m:m + 1])

                if phase == "gemm":
                    continue
                # -- dendrite IIR: i_d = beta*i_d + u, fused scan per m-tile --
                for m in range(M):
                    um = u[:, m * BL * C_:(m + 1) * BL * C_]
                    um3 = um.rearrange("p (b c) -> p b c", c=C_)
                    d0_sb = d0p.tile([128, BL * C_], fp32, tag="d0sb")
                    nc.sync.dma_start(d0_sb[:], d0_d.ap()[m])
                    if c > 0:
                        # u[:, b, 0] += beta * carry_b
                        nc.vector.scalar_tensor_tensor(
                            um3[:, :, 0], carry[:, m * BL:(m + 1) * BL],
                            bt[:, m:m + 1], um3[:, :, 0], AL.mult, AL.add)
                    nc.vector.tensor_tensor_scan(
                        um[:], d0_sb[:], um[:], 0.0, AL.mult, AL.add)
                    nc.scalar.copy(carry[:, m * BL:(m + 1) * BL],
                                   um3[:, :, C_ - 1])

                # -- branch sum + (1-alpha): w_j = (1-a)*(i_d[j] + i_d[2+j]) --
                # wti stored t-major: col = t*NF + j*BL + b, so the spike loop
                # reads contiguous (128, NF) slices per timestep.
                wre = wti[:].rearrange("p (c j b) -> p b j c", j=NJ, b=BL)
                for j in range(NJ):
                    wj = wre[:, :, j, :]                  # (128, BL, C) strided
                    nc.vector.tensor_tensor(
                        wj, u[:, j * BL * C_:(j + 1) * BL * C_].rearrange(
                            "p (b c) -> p b c", c=C_),
                        u[:, (2 + j) * BL * C_:(3 + j) * BL * C_].rearrange(
                            "p (b c) -> p b c", c=C_), AL.add)
                    nc.scalar.activation(wj, wj, AF.Identity,
                                         bias=0.0, scale=a1c[:, j:j + 1])

                if phase == "nov":
                    continue
                # -- spike loop (negated state: vt = -v) --
                for t in range(C_):
                    tA = hist[:, t * NF:(t + 1) * NF]   # pre-reset vt' kept
                    wt_t = wti[:, t * NF:(t + 1) * NF]
                    if alpha_uniform_val is not None:
                        # vt' = alpha*vt - w_t
                        nc.vector.scalar_tensor_tensor(
                            tA, vst[:], float(alpha_uniform_val),
                            wt_t, AL.mult, AL.subtract)
                    else:
                        nc.vector.tensor_tensor(tA, vst[:], atile[:], AL.mult)
                        nc.vector.tensor_tensor(tA, tA, wt_t, AL.subtract)
                    # vt'' = (vt' <= -1) + vt'   (spike subtract, negated)
                    nc.vector.scalar_tensor_tensor(
                        vst[:], tA, -1.0, tA, AL.is_le, AL.add)
                # counts += sum_t (vt'_t <= -1): compare in place, t-reduce
                nc.vector.tensor_scalar(hist[:], hist[:], -1.0, None, AL.is_le)
                csc = scr.tile([128, NF], fp32, tag="csc")
                nc.vector.tensor_reduce(
                    csc[:], hist[:].rearrange("p (c f) -> p f c", f=NF),
                    mybir.AxisListType.X, AL.add)
                nc.vector.tensor_tensor(counts[:], counts[:], csc[:], AL.add)

            # -- readout: out = W_out @ counts + b_out --
            po = pso.tile([D_OUT, BL], fp32, tag="po")
            nc.tensor.matmul(po[:], woutT_sb[:, 0:D_OUT], counts[:, 0:BL],
                             start=True, stop=False)
            nc.tensor.matmul(po[:], woutT_sb[0:H - 128, D_OUT:2 * D_OUT],
                             counts[0:H - 128, BL:2 * BL], start=False,
                             stop=True)
            out_sb = scr.tile([D_OUT, BL], fp32, tag="osb")
            nc.scalar.activation(out_sb[:], po[:], AF.Identity,
                                 bias=bout_sb[:, 0:1], scale=1.0)
            nc.sync.dma_start(out_d.ap(), out_sb[:])
            nc.sync.dma_start(counts_d.ap(), counts[:])

        if reps == 1:
            body_once()
        else:
            with tc.For_i(0, reps, 1):
                body_once()
        tok_sb = scr.tile([1, 16], fp32, tag="tok")
        nc.sync.dma_start(tok_sb[:], tok_d.ap())
        nc.sync.dma_start(tok_o.ap(), tok_sb[:])

    nc.compile()
    return nc


def _prep_host(x, W_in, b_in, tau_n, tau_m, W_out, b_out, T_, C_,
               gemm="f16x3"):
    """Host-side constant prep. Returns (shared_inputs, per_core_x, alpha_uni)."""
    x = _f32(x); W_in = _f32(W_in); b_in = _f32(b_in)
    tau_n = _f32(tau_n); tau_m = _f32(tau_m)
    W_out = _f32(W_out); b_out = _f32(b_out)

    beta = _f32(1.0 / (1.0 + np.exp(-tau_n.astype(np.float64))))   # (H,BR)
    alpha = _f32(1.0 / (1.0 + np.exp(-tau_m.astype(np.float64))))  # (H,)
    one = np.float32(1.0)

    # m-tile map: m=(br,j) -> rows p: h = (m%2)*128+p, o = h*BR + br
    wt = np.zeros((NK, 128, OP), np.float32)
    sc2 = np.zeros((128, M), np.float32)
    b2 = np.zeros((128, M), np.float32)
    bt = np.zeros((128, M), np.float32)
    for m in range(M):
        br, j = m // 2, m % 2
        for p in range(128):
            h = j * 128 + p
            if h >= H:
                continue
            o = h * BR + br
            sc2[p, m] = one - beta[h, br]
            b2[p, m] = (one - beta[h, br]) * b_in[o]
            bt[p, m] = beta[h, br]
            wrow = np.zeros(DP, np.float32)
            wrow[:D_IN] = W_in[o]
            wt[:, :, m * 128 + p] = wrow.reshape(NK, 128)
    a1c = np.zeros((128, NJ), np.float32)
    atile = np.zeros((128, NJ * BL), np.float32)
    for j in range(NJ):
        for p in range(128):
            h = j * 128 + p
            if h >= H:
                continue
            a1c[p, j] = one - alpha[h]
            atile[p, j * BL:(j + 1) * BL] = alpha[h]
    d0 = np.zeros((M, 128, BL * C_), np.float32)
    for m in range(M):
        d0[m, :, :] = bt[:, m:m + 1]
        d0[m, :, 0::C_] = 0.0
    woutT = np.zeros((256, D_OUT), np.float32)
    woutT[:H, :] = W_out.T
    bout = b_out.reshape(D_OUT, 1)

    def _split16(a):
        hi = a.astype(np.float16)
        lo = (a - hi.astype(np.float32)).astype(np.float16)
        return np.stack([hi, lo])

    wt_in = wt[None] if gemm == "f32" else _split16(wt)
    shared = dict(wt=wt_in, sc2=sc2, b2=b2, bt=bt, a1c=a1c, atile=atile,
                  d0=d0, woutT=_f32(woutT), bout=_f32(bout))
    xts = []
    NCH, BG = T_ // C_, 4
    for core in range(NCORES):
        xl = x[core * BL:(core + 1) * BL, :T_, :]        # (BL,T,D_IN)
        xp_ = np.zeros((BL, T_, DP), np.float32)
        xp_[:, :, :D_IN] = xl
        # -> [c, g, p, (k,bi,t)]
        xt = xp_.reshape(BL // BG, BG, NCH, C_, NK, 128).transpose(
            2, 0, 5, 4, 1, 3)                            # (NCH,NG,128,NK,BG,C)
        xt = _f32(xt.reshape(NCH, BL // BG, 128, NK * BG * C_))
        xts.append(xt[None] if gemm == "f32" else _split16(xt))
    uni = float(alpha[0]) if np.all(alpha == alpha[0]) else None
    return shared, xts, uni


TRACE = False          # set by test harness for profiling runs
LAST_RESULT = None


def kernel(x, W_in, b_in, tau_n, tau_m, W_out, b_out):
    global LAST_RESULT
    from concourse.bass_utils import run_bass_kernel_spmd

    T_, C_ = T, 125
    shared, xts, uni = _prep_host(x, W_in, b_in, tau_n, tau_m, W_out, b_out,
                                  T_, C_)
    nc = _build(T_, C_, alpha_uniform_val=uni)
    tok = np.zeros((1, 16), np.float32)
    in_maps = [dict(shared, xt=xts[core], tok=tok) for core in range(NCORES)]
    res = run_bass_kernel_spmd(nc, in_maps, core_ids=list(range(NCORES)),
                               trace=TRACE)
    LAST_RESULT = res
    out = np.empty((B, D_OUT), np.float32)
    for core in range(NCORES):
        out[core * BL:(core + 1) * BL, :] = res.results[core]["out"].T
    return out



# revision 14
# speedup vs baseline: 1.3766x; 1.3766x over previous
"""Trainium2 Bass kernel for the dendritic-branch spiking FNN (DH_SFNN).

Model (per reference):
  branch_in = x @ W_in.T + b_in                  # (B,T,H*BR)
  per t:  i_d = beta*i_d + (1-beta)*branch_in_t  # beta = sigmoid(tau_n), (H,BR)
          v   = alpha*v + (1-alpha)*i_d.sum(br)  # alpha = sigmoid(tau_m), (H,)
          spike = (v >= 1); v -= spike; counts += spike
  out = counts @ W_out.T + b_out                 # (B,D_OUT)

v2 strategy: data-parallel over batch across 8 cores (32 rows each).
  - GEMM: 2-pass fp16 (Wh@xh + Wh@xl; x split hi/lo -- x-side residual is
    the error direction that matters for spike flips). (1-beta)(1-alpha)
    and bias folded into the PSUM->SBUF activation copy. u double-buffered
    so the PE never stalls across chunk boundaries.
  - dendrite IIR: fused tensor_tensor_scan per (m-tile, chunk), split
    between DVE and Pool engines; d0 multiplier resident in SBUF.
  - spike loop: ONE custom DVE instruction per timestep via the pre-state
    recurrence  t_next = alpha*(t + (t <= -1)) - w  (negated potential).
    Spike counts recovered in bulk: cnthist += (hist <= -1), then a single
    strided reduce at the end.
  - readout: small PE matmul against W_out.
"""

import sys

if "/opt/trn_rl_repo" not in sys.path:
    sys.path.insert(0, "/opt/trn_rl_repo")

from contextlib import ExitStack

import numpy as np

import concourse.bass as bass
import concourse.mybir as mybir
import concourse.tile as tile
from concourse import bacc
import concourse.dve_ops as dve_ops
from concourse.dve_spec import Spec, Src0, Src1, C0, C1

B, T, D_IN, H, BR, D_OUT = 256, 500, 700, 200, 2, 35
NCORES = 8
BL = B // NCORES          # local batch = 32
NK = 6                    # k-tiles; D_IN padded 700 -> 768
DP = NK * 128
M = 4                     # m-tiles, m=(br,j): h = (m%2)*128+p, o = h*BR + br
OP = M * 128              # padded GEMM output rows (512)
NJ = 2                    # h groups (j=0: h<128, j=1: h 128..199)
NF = NJ * BL              # spike-state width (64): col = j*BL + b

# ---- custom DVE op: one LIF timestep on the negated pre-state ------------
# t_next = alpha*(t + (t <= -1)) - w ; spike bit of step t is (t_t <= -1)
if not any(op.name == "LIF_STEP_ANT" for op in dve_ops.OPS):
    LIF_STEP_ANT = dve_ops.DveOp(
        "LIF_STEP_ANT",
        Spec(
            body=(Src0 + (Src0 <= C1)) * C0 - Src1,
            reference=lambda in0, in1, s0, s1, imm2: (
                (in0 + (in0 <= s1).astype(np.float32)).astype(np.float32) * s0
                - in1
            ).astype(np.float32),
        ),
        subdim=False,
        uops_sha={"v3": "303557fd67bb3b15", "v4": "66f928dcfea37c7e"},
    )
    dve_ops.OPS.append(LIF_STEP_ANT)
    dve_ops.CUSTOM_DVE_SPECS["LIF_STEP_ANT"] = LIF_STEP_ANT.spec
    dve_ops._SUB_OPCODE_FOR_NAME["LIF_STEP_ANT"] = (
        dve_ops._CUSTOM_DVE_ROW_BASE + len(dve_ops.OPS) - 1)
else:
    LIF_STEP_ANT = next(op for op in dve_ops.OPS if op.name == "LIF_STEP_ANT")


def _f32(a):
    return np.ascontiguousarray(a, dtype=np.float32)


def _build(T_, C_, alpha_uniform_val=None, bsum_on_pool=True,
           use_custom=True, phase="full"):
    """Build the single-core Bass program.

    alpha_uniform_val: python float when alpha is constant across neurons
    (single fused chain op over all 64 cols); else per-j chain ops.
    Pool engine supports only tensor_tensor/copy/reduce/memset (no
    InstTensorScalarPtr: no scans, no STT) -- scans and counting stay on DVE.
    """
    NCH = T_ // C_
    BG = BL // 4           # 8 batches per matmul group
    NG = BL // BG          # 4 groups
    NN = BG * C_           # matmul free dim (400)
    assert NN <= 512 and T_ % C_ == 0
    fp32 = mybir.dt.float32
    fp16 = mybir.dt.float16
    AF = mybir.ActivationFunctionType
    AL = mybir.AluOpType

    nc = bacc.Bacc("TRN2", target_bir_lowering=False, debug=False,
                   num_devices=NCORES)

    # x: [2(hi,lo), NCH, NG, 128, NK*BG*C]; W: hi only [NK, 128, OP]
    xt_d = nc.dram_tensor("xt", [2, NCH, NG, 128, NK * BG * C_], fp16,
                          kind="ExternalInput")
    wt_d = nc.dram_tensor("wt", [NK, 128, OP], fp16, kind="ExternalInput")
    sc2_d = nc.dram_tensor("sc2", [128, M], fp32, kind="ExternalInput")
    b2_d = nc.dram_tensor("b2", [128, M], fp32, kind="ExternalInput")
    bt_d = nc.dram_tensor("bt", [128, M], fp32, kind="ExternalInput")
    a1j_d = nc.dram_tensor("a1j", [128, NJ], fp32, kind="ExternalInput")
    d0_d = nc.dram_tensor("d0", [M, 128, BL * C_], fp32, kind="ExternalInput")
    woutT_d = nc.dram_tensor("woutT", [2 * 128, D_OUT], fp32, kind="ExternalInput")
    bout_d = nc.dram_tensor("bout", [D_OUT, 1], fp32, kind="ExternalInput")
    out_d = nc.dram_tensor("out", [D_OUT, BL], fp32, kind="ExternalOutput")
    counts_d = nc.dram_tensor("counts", [128, NF], fp32, kind="ExternalOutput")

    with tile.TileContext(nc) as tc, ExitStack() as ctx:
        const = ctx.enter_context(tc.tile_pool(name="const", bufs=1))
        st = ctx.enter_context(tc.tile_pool(name="state", bufs=1))
        up = ctx.enter_context(tc.tile_pool(name="ubuf", bufs=2))
        hp = ctx.enter_context(tc.tile_pool(name="hist", bufs=1))
        xp = ctx.enter_context(tc.tile_pool(name="xin", bufs=3))
        ps = ctx.enter_context(tc.tile_pool(name="psum", bufs=4, space="PSUM"))
        pso = ctx.enter_context(tc.tile_pool(name="psout", bufs=1, space="PSUM"))
        scr = ctx.enter_context(tc.tile_pool(name="scr", bufs=2))

        w_sb = const.tile([128, NK * OP], fp16, tag="wsb")
        nc.sync.dma_start(
            w_sb[:].rearrange("p (k o) -> p k o", k=NK),
            wt_d.ap().rearrange("k p o -> p k o"))
        sc2 = const.tile([128, M], fp32)
        nc.sync.dma_start(sc2[:], sc2_d.ap())
        b2 = const.tile([128, M], fp32)
        nc.sync.dma_start(b2[:], b2_d.ap())
        bt = const.tile([128, M], fp32)
        nc.sync.dma_start(bt[:], bt_d.ap())
        a1j = const.tile([128, NJ], fp32)
        nc.sync.dma_start(a1j[:], a1j_d.ap())
        d0_sb = const.tile([128, M * BL * C_], fp32, tag="d0sb")
        nc.sync.dma_start(
            d0_sb[:].rearrange("p (m x) -> p m x", m=M),
            d0_d.ap().rearrange("m p x -> p m x"))
        woutT_sb = const.tile([128, 2 * D_OUT], fp32)
        nc.sync.dma_start(woutT_sb[:, 0:D_OUT], woutT_d.ap()[0:128])
        nc.sync.dma_start(woutT_sb[:, D_OUT:2 * D_OUT], woutT_d.ap()[128:256])
        bout_sb = const.tile([D_OUT, 1], fp32)
        nc.sync.dma_start(bout_sb[:], bout_d.ap())

        counts = st.tile([128, NF], fp32, tag="cnt")
        carry = st.tile([128, M * BL], fp32, tag="carry")
        wti = st.tile([128, C_ * NF], fp32, tag="wti")
        # sign-sum accumulator: S = sum_t sign(h_t + 1); counts = (T - S)/2
        cnthist = st.tile([128, C_ * NF], fp32, tag="cnthist")
        nc.gpsimd.memset(cnthist[:], 0.0)
        ind = st.tile([128, C_ * NF], fp32, tag="ind")

        hist0 = hp.tile([128, (C_ + 1) * NF], fp32, tag="hist0")
        hist1 = hp.tile([128, (C_ + 1) * NF], fp32, tag="hist1")
        hists = [hist0, hist1]
        nc.vector.memset(hists[0][:, 0:NF], 0.0)

        for c in range(NCH):
            u = up.tile([128, M * BL * C_], fp32, tag="u")
            # -- GEMM: u[m, b, t] = scaled(x @ Wh') via 2 fp16 passes --
            for g in range(NG):
                x_sbs = []
                for s in range(2):
                    x_sb_s = xp.tile([128, NK * NN], fp16, tag=f"xsb{s}")
                    nc.sync.dma_start(x_sb_s[:], xt_d.ap()[s, c, g])
                    x_sbs.append(x_sb_s)
                for m in range(M):
                    pt = ps.tile([128, NN], fp32, tag="pt")
                    nmm = 2 * NK
                    i = 0
                    for xs in range(2):
                        for k in range(NK):
                            nc.tensor.matmul(
                                pt[:],
                                w_sb[:, k * OP + m * 128:k * OP + (m + 1) * 128],
                                x_sbs[xs][:, k * NN:(k + 1) * NN],
                                start=(i == 0), stop=(i == nmm - 1))
                            i += 1
                    nc.scalar.activation(
                        u[:, m * BL * C_ + g * NN:m * BL * C_ + (g + 1) * NN],
                        pt[:], AF.Identity,
                        bias=b2[:, m:m + 1], scale=sc2[:, m:m + 1])

            if phase == "gemm":
                continue

            # -- dendrite IIR: i_d = beta*i_d + u, one scan per m-tile --
            for m in range(M):
                um = u[:, m * BL * C_:(m + 1) * BL * C_]
                um3 = um.rearrange("p (b c) -> p b c", c=C_)
                if c > 0:
                    nc.vector.scalar_tensor_tensor(
                        um3[:, :, 0], carry[:, m * BL:(m + 1) * BL],
                        bt[:, m:m + 1], um3[:, :, 0], AL.mult, AL.add)
                nc.vector.tensor_tensor_scan(
                    um[:], d0_sb[:, m * BL * C_:(m + 1) * BL * C_], um[:],
                    0.0, AL.mult, AL.add)
                nc.scalar.copy(carry[:, m * BL:(m + 1) * BL],
                               um3[:, :, C_ - 1])

            # -- branch sum: wti[t, j, b] = i_d[j] + i_d[2+j] --
            wre = wti[:].rearrange("p (c j b) -> p b j c", j=NJ, b=BL)
            beng = nc.gpsimd if bsum_on_pool else nc.vector
            for j in range(NJ):
                beng.tensor_tensor(
                    wre[:, :, j, :],
                    u[:, j * BL * C_:(j + 1) * BL * C_].rearrange(
                        "p (b c) -> p b c", c=C_),
                    u[:, (2 + j) * BL * C_:(3 + j) * BL * C_].rearrange(
                        "p (b c) -> p b c", c=C_), AL.add)

            if phase == "nov":
                continue

            # -- spike chain: one custom DVE op per timestep --
            hist = hists[c % 2]
            if c > 0:
                nc.scalar.copy(hist[:, 0:NF],
                               hists[(c - 1) % 2][:, C_ * NF:(C_ + 1) * NF])
            for t in range(C_):
                ho = hist[:, (t + 1) * NF:(t + 2) * NF]
                hi = hist[:, t * NF:(t + 1) * NF]
                wt_t = wti[:, t * NF:(t + 1) * NF]
                if use_custom:
                    if alpha_uniform_val is not None:
                        nc.vector._custom_dve(
                            LIF_STEP_ANT, out=ho, in0=hi, in1=wt_t,
                            s0=float(alpha_uniform_val), s1=-1.0)
                    else:
                        for j in range(NJ):
                            sl = slice(j * BL, (j + 1) * BL)
                            nc.vector._custom_dve(
                                LIF_STEP_ANT, out=ho[:, sl], in0=hi[:, sl],
                                in1=wt_t[:, sl],
                                s0=a1j[:, j:j + 1], s1=-1.0)
                else:
                    # stock 2-op fallback: post = hi + (hi<=-1); ho = a*post - w
                    tmp = scr.tile([128, NF], fp32, tag="tmp")
                    nc.vector.scalar_tensor_tensor(
                        tmp[:], hi, -1.0, hi, AL.is_le, AL.add)
                    nc.vector.scalar_tensor_tensor(
                        ho, tmp[:], float(alpha_uniform_val), wt_t,
                        AL.mult, AL.subtract)

            # -- spike counting: Act computes sign(h+1), Pool accumulates --
            nc.scalar.activation(ind[:], hist[:, NF:(C_ + 1) * NF],
                                 AF.Sign, bias=1.0, scale=1.0)
            nc.gpsimd.tensor_tensor(
                cnthist[:], cnthist[:], ind[:], AL.add)

        # -- S = sum_t sign-sums; out = (-0.5*W_out) @ S + bout_eff --
        # (woutT/bout are pre-transformed host-side so out == W_out@counts+b)
        ssum = scr.tile([128, NF], fp32, tag="ssum")
        nc.vector.tensor_reduce(
            ssum[:], cnthist[:].rearrange("p (c f) -> p f c", f=NF),
            mybir.AxisListType.X, AL.add)
        po = pso.tile([D_OUT, BL], fp32, tag="po")
        nc.tensor.matmul(po[:], woutT_sb[:, 0:D_OUT], ssum[:, 0:BL],
                         start=True, stop=False)
        nc.tensor.matmul(po[:], woutT_sb[0:H - 128, D_OUT:2 * D_OUT],
                         ssum[0:H - 128, BL:2 * BL], start=False, stop=True)
        out_sb = scr.tile([D_OUT, BL], fp32, tag="osb")
        nc.scalar.activation(out_sb[:], po[:], AF.Identity,
                             bias=bout_sb[:, 0:1], scale=1.0)
        nc.sync.dma_start(out_d.ap(), out_sb[:])
        # debug/inspection output: counts = (T - S)/2
        nc.vector.tensor_scalar(counts[:], ssum[:], -0.5, float(T_) / 2,
                                AL.mult, AL.add)
        nc.sync.dma_start(counts_d.ap(), counts[:])

    nc.compile()
    return nc


def _prep_host(x, W_in, b_in, tau_n, tau_m, W_out, b_out, T_, C_):
    """Host-side constant prep. Returns (shared_inputs, per_core_x, alpha_uni)."""
    x = _f32(x); W_in = _f32(W_in); b_in = _f32(b_in)
    tau_n = _f32(tau_n); tau_m = _f32(tau_m)
    W_out = _f32(W_out); b_out = _f32(b_out)

    beta = _f32(1.0 / (1.0 + np.exp(-tau_n.astype(np.float64))))   # (H,BR)
    alpha = _f32(1.0 / (1.0 + np.exp(-tau_m.astype(np.float64))))  # (H,)
    one = np.float32(1.0)

    # m-tile map: m=(br,j) -> rows p: h = (m%2)*128+p, o = h*BR + br
    wt = np.zeros((NK, 128, OP), np.float32)
    sc2 = np.zeros((128, M), np.float32)
    b2 = np.zeros((128, M), np.float32)
    btm = np.zeros((128, M), np.float32)
    for m in range(M):
        br, j = m // 2, m % 2
        for p in range(128):
            h = j * 128 + p
            if h >= H:
                continue
            o = h * BR + br
            a1 = one - alpha[h]
            sc2[p, m] = (one - beta[h, br]) * a1
            b2[p, m] = (one - beta[h, br]) * a1 * b_in[o]
            btm[p, m] = beta[h, br]
            wrow = np.zeros(DP, np.float32)
            wrow[:D_IN] = W_in[o]
            wt[:, :, m * 128 + p] = wrow.reshape(NK, 128)
    a1j = np.zeros((128, NJ), np.float32)
    for j in range(NJ):
        for p in range(128):
            h = j * 128 + p
            if h < H:
                a1j[p, j] = alpha[h]
    d0 = np.zeros((M, 128, BL * C_), np.float32)
    for m in range(M):
        d0[m, :, :] = btm[:, m:m + 1]
        d0[m, :, 0::C_] = 0.0
    # sign-sum readout transform: out = W_out@counts + b with
    # counts = (T - S)/2  ==>  out = (-W_out/2)@S + (b + (T/2)*W_out@1)
    woutT = np.zeros((256, D_OUT), np.float32)
    woutT[:H, :] = -0.5 * W_out.T
    bout = (b_out + (T_ / 2.0) * W_out.sum(axis=1)).reshape(D_OUT, 1)

    wt16 = wt.astype(np.float16)
    shared = dict(wt=wt16, sc2=sc2, b2=b2, bt=btm, a1j=a1j, d0=d0,
                  woutT=_f32(woutT), bout=_f32(bout))
    xts = []
    NCH, BG = T_ // C_, BL // 4
    for core in range(NCORES):
        xl = x[core * BL:(core + 1) * BL, :T_, :]        # (BL,T,D_IN)
        xp_ = np.zeros((BL, T_, DP), np.float32)
        xp_[:, :, :D_IN] = xl
        xt = xp_.reshape(BL // BG, BG, NCH, C_, NK, 128).transpose(
            2, 0, 5, 4, 1, 3)                     # (NCH,NG,128,NK,BG,C)
        xt = _f32(xt.reshape(NCH, BL // BG, 128, NK * BG * C_))
        hi = xt.astype(np.float16)
        lo = (xt - hi.astype(np.float32)).astype(np.float16)
        xts.append(np.stack([hi, lo]))
    uni = float(alpha[0]) if np.all(alpha == alpha[0]) else None
    return shared, xts, uni


TRACE = False          # set by test harness for profiling runs
LAST_RESULT = None


def kernel(x, W_in, b_in, tau_n, tau_m, W_out, b_out):
    global LAST_RESULT
    from concourse.bass_utils import run_bass_kernel_spmd

    T_, C_ = T, 50
    shared, xts, uni = _prep_host(x, W_in, b_in, tau_n, tau_m, W_out, b_out,
                                  T_, C_)
    nc = _build(T_, C_, alpha_uniform_val=uni)
    in_maps = [dict(shared, xt=xts[core]) for core in range(NCORES)]
    res = run_bass_kernel_spmd(nc, in_maps, core_ids=list(range(NCORES)),
                               trace=TRACE)
    LAST_RESULT = res
    out = np.empty((B, D_OUT), np.float32)
    for core in range(NCORES):
        out[core * BL:(core + 1) * BL, :] = res.results[core]["out"].T
    return out


# revision 20
# speedup vs baseline: 1.4367x; 1.0437x over previous
"""Trainium2 Bass kernel for the dendritic-branch spiking FNN (DH_SFNN).

Model (per reference):
  branch_in = x @ W_in.T + b_in                  # (B,T,H*BR)
  per t:  i_d = beta*i_d + (1-beta)*branch_in_t  # beta = sigmoid(tau_n), (H,BR)
          v   = alpha*v + (1-alpha)*i_d.sum(br)  # alpha = sigmoid(tau_m), (H,)
          spike = (v >= 1); v -= spike; counts += spike
  out = counts @ W_out.T + b_out                 # (B,D_OUT)

v2 strategy: data-parallel over batch across 8 cores (32 rows each).
  - GEMM: 2-pass fp16 (Wh@xh + Wh@xl; x split hi/lo -- x-side residual is
    the error direction that matters for spike flips). (1-beta)(1-alpha)
    and bias folded into the PSUM->SBUF activation copy. u double-buffered
    so the PE never stalls across chunk boundaries.
  - dendrite IIR: fused tensor_tensor_scan per (m-tile, chunk), split
    between DVE and Pool engines; d0 multiplier resident in SBUF.
  - spike loop: ONE custom DVE instruction per timestep via the pre-state
    recurrence  t_next = alpha*(t + (t <= -1)) - w  (negated potential).
    Spike counts recovered in bulk: cnthist += (hist <= -1), then a single
    strided reduce at the end.
  - readout: small PE matmul against W_out.
"""

import sys

if "/opt/trn_rl_repo" not in sys.path:
    sys.path.insert(0, "/opt/trn_rl_repo")

from contextlib import ExitStack

import numpy as np

import concourse.bass as bass
import concourse.mybir as mybir
import concourse.tile as tile
from concourse import bacc
import concourse.dve_ops as dve_ops
from concourse.dve_spec import Spec, Src0, Src1, C0, C1

B, T, D_IN, H, BR, D_OUT = 256, 500, 700, 200, 2, 35
NCORES = 8
BL = B // NCORES          # local batch = 32
NK = 6                    # k-tiles; D_IN padded 700 -> 768
DP = NK * 128
M = 4                     # m-tiles, m=(br,j): h = (m%2)*128+p, o = h*BR + br
OP = M * 128              # padded GEMM output rows (512)
NJ = 2                    # h groups (j=0: h<128, j=1: h 128..199)
NF = NJ * BL              # spike-state width (64): col = j*BL + b

# ---- custom DVE op: one LIF timestep on the negated pre-state ------------
# t_next = alpha*(t + (t <= -1)) - w ; spike bit of step t is (t_t <= -1)
if not any(op.name == "LIF_STEP_ANT" for op in dve_ops.OPS):
    LIF_STEP_ANT = dve_ops.DveOp(
        "LIF_STEP_ANT",
        Spec(
            body=(Src0 + (Src0 <= C1)) * C0 - Src1,
            reference=lambda in0, in1, s0, s1, imm2: (
                (in0 + (in0 <= s1).astype(np.float32)).astype(np.float32) * s0
                - in1
            ).astype(np.float32),
        ),
        subdim=False,
        uops_sha={"v3": "303557fd67bb3b15", "v4": "66f928dcfea37c7e"},
    )
    dve_ops.OPS.append(LIF_STEP_ANT)
    dve_ops.CUSTOM_DVE_SPECS["LIF_STEP_ANT"] = LIF_STEP_ANT.spec
    dve_ops._SUB_OPCODE_FOR_NAME["LIF_STEP_ANT"] = (
        dve_ops._CUSTOM_DVE_ROW_BASE + len(dve_ops.OPS) - 1)
else:
    LIF_STEP_ANT = next(op for op in dve_ops.OPS if op.name == "LIF_STEP_ANT")


def _f32(a):
    return np.ascontiguousarray(a, dtype=np.float32)


def _build(T_, C_, alpha_uniform_val=None,
           use_custom=True, phase="full"):
    """Build the single-core Bass program.

    alpha_uniform_val: python float when alpha is constant across neurons
    (single fused chain op over all 64 cols); else per-j chain ops.
    Pool engine supports only tensor_tensor/copy/reduce/memset (no
    InstTensorScalarPtr: no scans, no STT) -- scans and counting stay on DVE.
    """
    NCH = T_ // C_
    BG = BL // 4           # 8 batches per matmul group
    NG = BL // BG          # 4 groups
    NN = BG * C_           # matmul free dim (400)
    assert NN <= 512 and T_ % C_ == 0
    fp32 = mybir.dt.float32
    fp16 = mybir.dt.float16
    AF = mybir.ActivationFunctionType
    AL = mybir.AluOpType

    nc = bacc.Bacc("TRN2", target_bir_lowering=False, debug=False,
                   num_devices=NCORES)

    # x: [2(hi,lo), NCH, NG, 128, NK*BG*C]; W: hi only [NK, 128, OP]
    xt_d = nc.dram_tensor("xt", [2, NCH, NG, 128, NK * BG * C_], fp16,
                          kind="ExternalInput")
    wt_d = nc.dram_tensor("wt", [NK, 128, OP], fp16, kind="ExternalInput")
    sc2_d = nc.dram_tensor("sc2", [128, M], fp32, kind="ExternalInput")
    b2_d = nc.dram_tensor("b2", [128, M], fp32, kind="ExternalInput")
    bt_d = nc.dram_tensor("bt", [128, M], fp32, kind="ExternalInput")
    a1j_d = nc.dram_tensor("a1j", [128, NJ], fp32, kind="ExternalInput")
    d0_d = nc.dram_tensor("d0", [M, 128, BL * C_], fp32, kind="ExternalInput")
    woutT_d = nc.dram_tensor("woutT", [2 * 128, D_OUT], fp32, kind="ExternalInput")
    bout_d = nc.dram_tensor("bout", [D_OUT, 1], fp32, kind="ExternalInput")
    out_d = nc.dram_tensor("out", [D_OUT, BL], fp32, kind="ExternalOutput")
    counts_d = nc.dram_tensor("counts", [128, NF], fp32, kind="ExternalOutput")

    with tile.TileContext(nc) as tc, ExitStack() as ctx:
        const = ctx.enter_context(tc.tile_pool(name="const", bufs=1))
        st = ctx.enter_context(tc.tile_pool(name="state", bufs=1))
        up = ctx.enter_context(tc.tile_pool(name="ubuf", bufs=2))
        hp = ctx.enter_context(tc.tile_pool(name="hist", bufs=1))
        xp = ctx.enter_context(tc.tile_pool(name="xin", bufs=5))
        ps = ctx.enter_context(tc.tile_pool(name="psum", bufs=6, space="PSUM"))
        pso = ctx.enter_context(tc.tile_pool(name="psout", bufs=1, space="PSUM"))
        scr = ctx.enter_context(tc.tile_pool(name="scr", bufs=2))

        w_sb = const.tile([128, NK * OP], fp16, tag="wsb")
        nc.sync.dma_start(
            w_sb[:].rearrange("p (k o) -> p k o", k=NK),
            wt_d.ap().rearrange("k p o -> p k o"))
        sc2 = const.tile([128, M], fp32)
        nc.sync.dma_start(sc2[:], sc2_d.ap())
        b2 = const.tile([128, M], fp32)
        nc.sync.dma_start(b2[:], b2_d.ap())
        bt = const.tile([128, M], fp32)
        nc.sync.dma_start(bt[:], bt_d.ap())
        a1j = const.tile([128, NJ], fp32)
        nc.sync.dma_start(a1j[:], a1j_d.ap())
        d0_sb = const.tile([128, M * BL * C_], fp32, tag="d0sb")
        nc.sync.dma_start(
            d0_sb[:].rearrange("p (m x) -> p m x", m=M),
            d0_d.ap().rearrange("m p x -> p m x"))
        woutT_sb = const.tile([128, 2 * D_OUT], fp32)
        nc.sync.dma_start(woutT_sb[:, 0:D_OUT], woutT_d.ap()[0:128])
        nc.sync.dma_start(woutT_sb[:, D_OUT:2 * D_OUT], woutT_d.ap()[128:256])
        bout_sb = const.tile([D_OUT, 1], fp32)
        nc.sync.dma_start(bout_sb[:], bout_d.ap())

        counts = st.tile([128, NF], fp32, tag="cnt")
        carry = st.tile([128, M * BL], fp32, tag="carry")
        wti0 = st.tile([128, C_ * NF], fp32, tag="wti0")
        wti1 = st.tile([128, C_ * NF], fp32, tag="wti1")
        wtis = [wti0, wti1]
        # sign-sum accumulator: S = sum_t sign(h_t + 1); counts = (T - S)/2
        cnthist = st.tile([128, C_ * NF], fp32, tag="cnthist")
        nc.gpsimd.memset(cnthist[:], 0.0)
        ind = st.tile([128, C_ * NF], fp32, tag="ind")

        hist0 = hp.tile([128, (C_ + 1) * NF], fp32, tag="hist0")
        hist1 = hp.tile([128, (C_ + 1) * NF], fp32, tag="hist1")
        hists = [hist0, hist1]
        nc.vector.memset(hists[0][:, 0:NF], 0.0)

        def count_chunk(cc):
            # sign(h+1) on Act (off the u-copy critical path: deferred one
            # chunk), sum into cnthist on Pool
            nc.scalar.activation(ind[:], hists[cc % 2][:, NF:(C_ + 1) * NF],
                                 AF.Sign, bias=1.0, scale=1.0)
            nc.gpsimd.tensor_tensor(
                cnthist[:], cnthist[:], ind[:], AL.add)

        for c in range(NCH):
            u = up.tile([128, M * BL * C_], fp32, tag="u")
            # -- GEMM: u[m, b, t] = scaled(x @ Wh') via 2 fp16 passes --
            for g in range(NG):
                x_sbs = []
                for s in range(2):
                    x_sb_s = xp.tile([128, NK * NN], fp16, tag=f"xsb{s}")
                    nc.sync.dma_start(x_sb_s[:], xt_d.ap()[s, c, g])
                    x_sbs.append(x_sb_s)
                for m in range(M):
                    pt = ps.tile([128, NN], fp32, tag="pt")
                    nmm = 2 * NK
                    i = 0
                    for xs in range(2):
                        for k in range(NK):
                            nc.tensor.matmul(
                                pt[:],
                                w_sb[:, k * OP + m * 128:k * OP + (m + 1) * 128],
                                x_sbs[xs][:, k * NN:(k + 1) * NN],
                                start=(i == 0), stop=(i == nmm - 1))
                            i += 1
                    nc.scalar.activation(
                        u[:, m * BL * C_ + g * NN:m * BL * C_ + (g + 1) * NN],
                        pt[:], AF.Identity,
                        bias=b2[:, m:m + 1], scale=sc2[:, m:m + 1])

            if phase == "gemm":
                continue

            # deferred spike counting for the previous chunk (keeps the Act
            # queue free of DVE-dependent work ahead of u-copies)
            if c > 0:
                count_chunk(c - 1)

            # -- dendrite IIR scans (per m-tile) + branch sums, j-grouped so
            #    each bsum follows its two scans immediately on DVE --
            wti = wtis[c % 2]
            wre = wti[:].rearrange("p (c j b) -> p b j c", j=NJ, b=BL)
            for j in range(NJ):
                for m in (j, 2 + j):
                    um = u[:, m * BL * C_:(m + 1) * BL * C_]
                    um3 = um.rearrange("p (b c) -> p b c", c=C_)
                    if c > 0:
                        nc.vector.scalar_tensor_tensor(
                            um3[:, :, 0], carry[:, m * BL:(m + 1) * BL],
                            bt[:, m:m + 1], um3[:, :, 0], AL.mult, AL.add)
                    nc.vector.tensor_tensor_scan(
                        um[:], d0_sb[:, m * BL * C_:(m + 1) * BL * C_], um[:],
                        0.0, AL.mult, AL.add)
                    nc.vector.tensor_copy(carry[:, m * BL:(m + 1) * BL],
                                          um3[:, :, C_ - 1])
                nc.vector.tensor_tensor(
                    wre[:, :, j, :],
                    u[:, j * BL * C_:(j + 1) * BL * C_].rearrange(
                        "p (b c) -> p b c", c=C_),
                    u[:, (2 + j) * BL * C_:(3 + j) * BL * C_].rearrange(
                        "p (b c) -> p b c", c=C_), AL.add)

            if phase == "nov":
                continue

            # -- spike chain: one custom DVE op per timestep --
            hist = hists[c % 2]
            if c > 0:
                nc.vector.tensor_copy(
                    hist[:, 0:NF],
                    hists[(c - 1) % 2][:, C_ * NF:(C_ + 1) * NF])
            for t in range(C_):
                ho = hist[:, (t + 1) * NF:(t + 2) * NF]
                hi = hist[:, t * NF:(t + 1) * NF]
                wt_t = wti[:, t * NF:(t + 1) * NF]
                if use_custom:
                    if alpha_uniform_val is not None:
                        nc.vector._custom_dve(
                            LIF_STEP_ANT, out=ho, in0=hi, in1=wt_t,
                            s0=float(alpha_uniform_val), s1=-1.0)
                    else:
                        for j in range(NJ):
                            sl = slice(j * BL, (j + 1) * BL)
                            nc.vector._custom_dve(
                                LIF_STEP_ANT, out=ho[:, sl], in0=hi[:, sl],
                                in1=wt_t[:, sl],
                                s0=a1j[:, j:j + 1], s1=-1.0)
                else:
                    # stock 2-op fallback: post = hi + (hi<=-1); ho = a*post - w
                    tmp = scr.tile([128, NF], fp32, tag="tmp")
                    nc.vector.scalar_tensor_tensor(
                        tmp[:], hi, -1.0, hi, AL.is_le, AL.add)
                    nc.vector.scalar_tensor_tensor(
                        ho, tmp[:], float(alpha_uniform_val), wt_t,
                        AL.mult, AL.subtract)

        count_chunk(NCH - 1)

        # -- S = sum_t sign-sums; out = (-0.5*W_out) @ S + bout_eff --
        # (woutT/bout are pre-transformed host-side so out == W_out@counts+b)
        ssum = scr.tile([128, NF], fp32, tag="ssum")
        nc.vector.tensor_reduce(
            ssum[:], cnthist[:].rearrange("p (c f) -> p f c", f=NF),
            mybir.AxisListType.X, AL.add)
        po = pso.tile([D_OUT, BL], fp32, tag="po")
        nc.tensor.matmul(po[:], woutT_sb[:, 0:D_OUT], ssum[:, 0:BL],
                         start=True, stop=False)
        nc.tensor.matmul(po[:], woutT_sb[0:H - 128, D_OUT:2 * D_OUT],
                         ssum[0:H - 128, BL:2 * BL], start=False, stop=True)
        out_sb = scr.tile([D_OUT, BL], fp32, tag="osb")
        nc.scalar.activation(out_sb[:], po[:], AF.Identity,
                             bias=bout_sb[:, 0:1], scale=1.0)
        nc.sync.dma_start(out_d.ap(), out_sb[:])
        # debug/inspection output: counts = (T - S)/2
        nc.vector.tensor_scalar(counts[:], ssum[:], -0.5, float(T_) / 2,
                                AL.mult, AL.add)
        nc.sync.dma_start(counts_d.ap(), counts[:])

    nc.compile()
    return nc


def _prep_host(x, W_in, b_in, tau_n, tau_m, W_out, b_out, T_, C_):
    """Host-side constant prep. Returns (shared_inputs, per_core_x, alpha_uni)."""
    x = _f32(x); W_in = _f32(W_in); b_in = _f32(b_in)
    tau_n = _f32(tau_n); tau_m = _f32(tau_m)
    W_out = _f32(W_out); b_out = _f32(b_out)

    beta = _f32(1.0 / (1.0 + np.exp(-tau_n.astype(np.float64))))   # (H,BR)
    alpha = _f32(1.0 / (1.0 + np.exp(-tau_m.astype(np.float64))))  # (H,)
    one = np.float32(1.0)

    # m-tile map: m=(br,j) -> rows p: h = (m%2)*128+p, o = h*BR + br
    wt = np.zeros((NK, 128, OP), np.float32)
    sc2 = np.zeros((128, M), np.float32)
    b2 = np.zeros((128, M), np.float32)
    btm = np.zeros((128, M), np.float32)
    for m in range(M):
        br, j = m // 2, m % 2
        for p in range(128):
            h = j * 128 + p
            if h >= H:
                continue
            o = h * BR + br
            a1 = one - alpha[h]
            sc2[p, m] = (one - beta[h, br]) * a1
            b2[p, m] = (one - beta[h, br]) * a1 * b_in[o]
            btm[p, m] = beta[h, br]
            wrow = np.zeros(DP, np.float32)
            wrow[:D_IN] = W_in[o]
            wt[:, :, m * 128 + p] = wrow.reshape(NK, 128)
    a1j = np.zeros((128, NJ), np.float32)
    for j in range(NJ):
        for p in range(128):
            h = j * 128 + p
            if h < H:
                a1j[p, j] = alpha[h]
    d0 = np.zeros((M, 128, BL * C_), np.float32)
    for m in range(M):
        d0[m, :, :] = btm[:, m:m + 1]
        d0[m, :, 0::C_] = 0.0
    # sign-sum readout transform: out = W_out@counts + b with
    # counts = (T - S)/2  ==>  out = (-W_out/2)@S + (b + (T/2)*W_out@1)
    woutT = np.zeros((256, D_OUT), np.float32)
    woutT[:H, :] = -0.5 * W_out.T
    bout = (b_out + (T_ / 2.0) * W_out.sum(axis=1)).reshape(D_OUT, 1)

    wt16 = wt.astype(np.float16)
    shared = dict(wt=wt16, sc2=sc2, b2=b2, bt=btm, a1j=a1j, d0=d0,
                  woutT=_f32(woutT), bout=_f32(bout))
    xts = []
    NCH, BG = T_ // C_, BL // 4
    for core in range(NCORES):
        xl = x[core * BL:(core + 1) * BL, :T_, :]        # (BL,T,D_IN)
        xp_ = np.zeros((BL, T_, DP), np.float32)
        xp_[:, :, :D_IN] = xl
        xt = xp_.reshape(BL // BG, BG, NCH, C_, NK, 128).transpose(
            2, 0, 5, 4, 1, 3)                     # (NCH,NG,128,NK,BG,C)
        xt = _f32(xt.reshape(NCH, BL // BG, 128, NK * BG * C_))
        hi = xt.astype(np.float16)
        lo = (xt - hi.astype(np.float32)).astype(np.float16)
        xts.append(np.stack([hi, lo]))
    uni = float(alpha[0]) if np.all(alpha == alpha[0]) else None
    return shared, xts, uni


TRACE = False          # set by test harness for profiling runs
LAST_RESULT = None


def kernel(x, W_in, b_in, tau_n, tau_m, W_out, b_out):
    global LAST_RESULT
    from concourse.bass_utils import run_bass_kernel_spmd

    T_, C_ = T, 50
    shared, xts, uni = _prep_host(x, W_in, b_in, tau_n, tau_m, W_out, b_out,
                                  T_, C_)
    nc = _build(T_, C_, alpha_uniform_val=uni)
    in_maps = [dict(shared, xt=xts[core]) for core in range(NCORES)]
    res = run_bass_kernel_spmd(nc, in_maps, core_ids=list(range(NCORES)),
                               trace=TRACE)
    LAST_RESULT = res
    out = np.empty((B, D_OUT), np.float32)
    for core in range(NCORES):
        out[core * BL:(core + 1) * BL, :] = res.results[core]["out"].T
    return out
